# revision 9
# baseline (speedup 1.0000x reference)
"""BERT self-attention (B=4, S=2048, H=768, 12 heads x 64) on 8 trn2 cores.

Sharding: core c = batch (c//2) x head-half (c%2, 6 heads each).
Each core computes Q/K/V projections for its 6 heads, attention, and a
partial output projection (its heads' slice of Wo). Host sums the two
partials per batch and adds bo.

On-device layout (per core):
  xT   [768, 2048]  bf16  (DMA-transposed x)
  QT/KT per head-pair [128=2x64, 2048] bf16   (head-dim on partitions)
  V    16 tiles [128 keys, 6 heads x 65] bf16 (col 64 of each head = 1.0)
  scores^T [128 keys, 2x512 q] fp32 PSUM (two heads packed via row tiling)
  exp on ScalarE (scale=1/8, bias=mask column), out bf16
  attn@V -> comb [65, 512] PSUM; row 64 = softmax denominator
  combT stacked per head pair [128 = 2x64 dims, 2048] bf16 (head B half
  placed via a small SBUF->SBUF DMA hop), scaled by 1/denom
  out-proj: 3 x K=128 chunks accumulated in PSUM, DVE-drained, DMA out

The attention loop is PE-bound overall; ACT(exp) is a close second.
DMAs are ordered so the first scores (wk, wq, x piece 0) land first,
pulling the first exp to ~8us; V projection and leftover Q/K chains are
injected into the exp-wait bubbles of the early slots instead of
running in a DMA-starved preamble.
"""

import numpy as np
import ml_dtypes

B, S, H = 4, 2048, 768
NH, HS = 12, 64
NHL = 6              # heads per core
NHP = 3              # head pairs per core
HCHUNKS = 6          # 768 / 128 contraction chunks
SKT = 16             # key tiles of 128
SQT = 4              # query tiles of 512
QW = 512             # query tile width
N_CORES = 8

_COMPILED = None


def _build():
    import concourse.bass as bass
    import concourse.mybir as mybir
    import concourse.tile as tile
    from concourse import bacc

    fp32 = mybir.dt.float32
    bf16 = mybir.dt.bfloat16
    AF = mybir.ActivationFunctionType

    nc = bacc.Bacc("TRN2", target_bir_lowering=False, debug=False)

    xt_d = nc.dram_tensor("xt", [H, S], bf16, kind="ExternalInput").ap()
    wq_d = nc.dram_tensor("wq", [H, NHL * HS], bf16, kind="ExternalInput").ap()
    wk_d = nc.dram_tensor("wk", [H, NHL * HS], bf16, kind="ExternalInput").ap()
    wv_d = nc.dram_tensor("wv", [H, NHL * HS], bf16, kind="ExternalInput").ap()
    wo_d = nc.dram_tensor("wo", [NHL * HS, H], bf16, kind="ExternalInput").ap()
    bq_d = nc.dram_tensor("bq", [128, NHP], fp32, kind="ExternalInput").ap()
    bk_d = nc.dram_tensor("bk", [128, NHP], fp32, kind="ExternalInput").ap()
    bv_d = nc.dram_tensor("bv", [128, NHL * HS], fp32, kind="ExternalInput").ap()
    mask_d = nc.dram_tensor("mask", [128, SKT], fp32, kind="ExternalInput").ap()
    out_d = nc.dram_tensor("out", [S, H], fp32, kind="ExternalOutput").ap()

    with tile.TileContext(nc) as tc:
        with (
            tc.tile_pool(name="const", bufs=1) as const,
            tc.tile_pool(name="xt", bufs=1) as xtp,
            tc.tile_pool(name="vsb", bufs=1) as vsb,
            tc.tile_pool(name="qkt", bufs=2) as qkt,
            tc.tile_pool(name="combt", bufs=1) as combtp,
            tc.tile_pool(name="attn", bufs=5) as attnp,
            tc.tile_pool(name="small", bufs=4) as smallp,
            tc.tile_pool(name="outsb", bufs=4) as outsb,
            tc.tile_pool(name="ps_sc", bufs=2, space="PSUM") as ps_sc,
            tc.tile_pool(name="ps_cb", bufs=2, space="PSUM") as ps_cb,
            tc.tile_pool(name="ps_pj", bufs=2, space="PSUM") as ps_pj,
        ):
            # ---- tiles ----
            xt = [[None] * SQT for _ in range(HCHUNKS)]
            for piece in range(SQT):
                for c in range(HCHUNKS):
                    t = xtp.tile([128, QW], bf16, tag=f"xt{c}_{piece}",
                                 name=f"xt{c}_{piece}")
                    xt[c][piece] = t
            wv_sb, wq_sb, wk_sb = [], [], []
            for c in range(HCHUNKS):
                wv_sb.append(const.tile([128, NHL * HS], bf16, tag=f"wv{c}", name=f"wv{c}"))
                wq_sb.append(const.tile([128, NHL * HS], bf16, tag=f"wq{c}", name=f"wq{c}"))
                wk_sb.append(const.tile([128, NHL * HS], bf16, tag=f"wk{c}", name=f"wk{c}"))
            bq_sb = const.tile([128, NHP], fp32, tag="bq")
            bk_sb = const.tile([128, NHP], fp32, tag="bk")
            bv_sb = const.tile([128, NHL * HS], fp32, tag="bv")
            mask_sb = const.tile([128, SKT], fp32, tag="mask")
            wo_sb = [const.tile([128, H], bf16, tag=f"wo{hp}", name=f"wo{hp}")
                     for hp in range(NHP)]

            # ---- DMA emission: first-exp dependencies first.
            # sync: x pieces (+mask); scalar: all weights in consumption
            # order (wk/wq gate the first scores, wv the first attn@V,
            # wo only the hp2 phase). gpsimd queue is kept for runtime
            # SBUF->SBUF hops.
            for c in range(HCHUNKS):
                nc.sync.dma_start(xt[c][0][:], xt_d[c * 128:(c + 1) * 128, 0:QW])
            for c in range(HCHUNKS):
                nc.scalar.dma_start(wk_sb[c][:], wk_d[c * 128:(c + 1) * 128, :])
            nc.scalar.dma_start(bk_sb[:], bk_d[:])
            for c in range(HCHUNKS):
                nc.scalar.dma_start(wq_sb[c][:], wq_d[c * 128:(c + 1) * 128, :])
            nc.scalar.dma_start(bq_sb[:], bq_d[:])
            nc.sync.dma_start(mask_sb[:], mask_d[:])
            for c in range(HCHUNKS):
                nc.scalar.dma_start(wv_sb[c][:], wv_d[c * 128:(c + 1) * 128, :])
            nc.scalar.dma_start(bv_sb[:], bv_d[:])
            for c in range(HCHUNKS):
                nc.sync.dma_start(xt[c][1][:], xt_d[c * 128:(c + 1) * 128, QW:2 * QW])
            for c in range(HCHUNKS):
                nc.gpsimd.dma_start(xt[c][2][:], xt_d[c * 128:(c + 1) * 128, 2 * QW:3 * QW])
            for hp in range(NHP):
                nc.scalar.dma_start(wo_sb[hp][:], wo_d[hp * 128:(hp + 1) * 128, :])
            for c in range(HCHUNKS):
                nc.gpsimd.dma_start(xt[c][3][:], xt_d[c * 128:(c + 1) * 128, 3 * QW:4 * QW])

            # ---- V projection: V[s, h*65+d], col h*65+64 = 1.0.
            # Emitted as per-kt unit chains, all injected into the
            # attention loop's exp-wait bubbles. ----
            v_sb = [vsb.tile([128, NHL, HS + 1], bf16, tag=f"v{kt}", name=f"v{kt}")
                    for kt in range(SKT)]

            def v_units(kt):
                vt = v_sb[kt]
                st8 = {}

                def unit(c, st8=st8):
                    if c == 0:
                        st8["ps"] = ps_pj.tile([128, 512], fp32, tag="pj",
                                               name="psv")
                    nc.tensor.matmul(
                        st8["ps"][:, :NHL * HS],
                        lhsT=xt[c][kt // 4][:, (kt % 4) * 128:(kt % 4 + 1) * 128],
                        rhs=wv_sb[c][:],
                        start=(c == 0),
                        stop=(c == HCHUNKS - 1),
                    )
                    if c == HCHUNKS - 1:
                        nc.vector.tensor_add(
                            vt[:, :, 0:HS],
                            st8["ps"][:, :NHL * HS].rearrange(
                                "p (h d) -> p h d", h=NHL),
                            bv_sb[:].rearrange("p (h d) -> p h d", h=NHL),
                        )
                        nc.vector.memset(vt[:, :, HS:HS + 1], 1.0)

                return [lambda c=c: unit(c) for c in range(HCHUNKS)]

            # combT stacked per head pair: rows 0-63 = head 2hp,
            # rows 64-127 = head 2hp+1 (written via DMA hop).
            combt_s = [combtp.tile([128, S], bf16, tag=f"cts{hp}", name=f"cts{hp}")
                       for hp in range(NHP)]

            def emit_qkt(hp):
                """Q^T/K^T projection for head pair hp; returns (qt, kt, units).
                Each unit emits one matmul (plus bias-add drain on the last)."""
                qt_t = qkt.tile([128, S], bf16, tag="qt", name=f"qt{hp}")
                kt_t = qkt.tile([128, S], bf16, tag="kt", name=f"kt{hp}")
                units = []
                chains = {}
                for kind, dst, w_sb, b_sb in (("kt", kt_t, wk_sb, bk_sb),
                                              ("qt", qt_t, wq_sb, bq_sb)):
                    for sq in range(SQT):
                        st8 = {}

                        def unit(c, dst=dst, w_sb=w_sb, b_sb=b_sb, sq=sq, st8=st8):
                            if c == 0:
                                st8["ps"] = ps_pj.tile(
                                    [128, 512], fp32, tag="pj", name="psq")
                            nc.tensor.matmul(
                                st8["ps"][:],
                                lhsT=w_sb[c][:, hp * 128:(hp + 1) * 128],
                                rhs=xt[c][sq][:],
                                start=(c == 0),
                                stop=(c == HCHUNKS - 1),
                            )
                            if c == HCHUNKS - 1:
                                nc.vector.tensor_scalar_add(
                                    dst[:, sq * QW:(sq + 1) * QW], st8["ps"][:],
                                    b_sb[:, hp:hp + 1],
                                )

                        chain = [lambda c=c, u=unit: u(c)
                                 for c in range(HCHUNKS)]
                        chains[(kind, sq)] = chain
                        units.extend(chain)
                return qt_t, kt_t, units, chains

            def emit_outproj_unit(st, half, hpc, st8):
                """One of three K=128 matmuls of the out-proj psum chain
                (st, half); hpc = head-pair chunk. Last chunk drains to
                SBUF and streams the result out."""
                if hpc == 0:
                    st8["ps"] = ps_pj.tile([128, 512], fp32, tag="pj", name="pso")
                nc.tensor.matmul(
                    st8["ps"][:, 0:384],
                    lhsT=combt_s[hpc][:, st * 128:(st + 1) * 128],
                    rhs=wo_sb[hpc][:, half * 384:(half + 1) * 384],
                    start=(hpc == 0), stop=(hpc == NHP - 1),
                )
                if hpc == NHP - 1:
                    ob = outsb.tile([128, 384], fp32, tag="ob")
                    nc.vector.tensor_copy(ob[:], st8["ps"][:, 0:384])
                    nc.sync.dma_start(
                        out_d[st * 128:(st + 1) * 128,
                              half * 384:(half + 1) * 384], ob[:])

            def outproj_units(sqs, min_sq):
                """(min_sq, deadline, unit) out-proj chains for the s-tiles
                inside query tiles `sqs` (needs all 3 head pairs' combT)."""
                units = []
                for sq in sqs:
                    for st in range(4 * sq, 4 * (sq + 1)):
                        for half in range(2):
                            st8 = {}
                            for hpc in range(NHP):
                                units.append((min_sq, None,
                                              lambda st=st, half=half,
                                              hpc=hpc, st8=st8:
                                              emit_outproj_unit(st, half, hpc, st8)))
                return units

            # ---- minimal preamble: only what the first two scores need ----
            qkts = [emit_qkt(0)]
            ch0 = qkts[0][3]
            for u in ch0[("kt", 0)] + ch0[("qt", 0)] + ch0[("kt", 1)]:
                u()

            # per-hp injection queues: (min_sq, deadline, emit_fn).
            # deadline = global slot index by whose lookahead-emission the
            # unit must have been EMITTED (its output is read by that
            # slot's scores lookahead or attn@V); None = no deadline.
            inject_q = {0: [], 1: [], 2: []}
            qkts.append(emit_qkt(1))
            inject_q[0] = (
                [(0, kt, u) for kt in range(0, 2) for u in v_units(kt)]
                + [(0, 8, u) for u in ch0[("kt", 2)]]
                + [(0, kt, u) for kt in range(2, 4) for u in v_units(kt)]
                + [(0, 12, u) for u in ch0[("kt", 3)]]
                + [(0, kt, u) for kt in range(4, 6) for u in v_units(kt)]
                + [(0, 16, u) for u in ch0[("qt", 1)]]
                + [(0, kt, u) for kt in range(6, SKT) for u in v_units(kt)]
                + [(1, 32, u) for u in ch0[("qt", 2)]]
                + [(2, 48, u) for u in ch0[("qt", 3)]]
                + [(0, 64, u) for u in qkts[1][2]])

            slots = [(hp, sq, kt) for hp in range(NHP) for sq in range(SQT)
                     for kt in range(SKT)]

            def scores(hp, sq, kt):
                qt_t, kt_t = qkts[hp][0], qkts[hp][1]
                sc = ps_sc.tile([128, 1024], fp32, tag="sc", name="sc")
                nc.tensor.matmul(
                    sc[:, 0:512],
                    lhsT=kt_t[0:64, kt * 128:(kt + 1) * 128],
                    rhs=qt_t[0:64, sq * QW:(sq + 1) * QW],
                    start=True, stop=True,
                )
                nc.tensor.matmul(
                    sc[:, 512:1024],
                    lhsT=kt_t[64:128, kt * 128:(kt + 1) * 128],
                    rhs=qt_t[64:128, sq * QW:(sq + 1) * QW],
                    start=True, stop=True,
                )
                return sc

            cb_cur = None
            sc_cur = scores(*slots[0])
            for i, (hp, sq, kt) in enumerate(slots):
                if kt == 0:
                    if sq == 0 and hp > 0:
                        # drain any leftover injected work of the previous hp
                        for _, _, u in inject_q[hp - 1]:
                            u()
                        inject_q[hp - 1] = []
                    # build hp-level injection queues lazily at hp start
                    if sq == 0 and hp == 1:
                        qkts.append(emit_qkt(2))
                        inject_q[1] = [(0, 128, u) for u in qkts[2][2]]
                    if sq == 0 and hp == 2:
                        inject_q[2] = (outproj_units([0], min_sq=1)
                                       + outproj_units([1], min_sq=2)
                                       + outproj_units([2], min_sq=3))
                    cb_a = ps_cb.tile([65, 512], fp32, tag="cb", name="cba")
                    cb_b = ps_cb.tile([65, 512], fp32, tag="cb", name="cbb")
                    cb_cur = (cb_a, cb_b)
                q = inject_q[hp]
                # forced pops: units whose output the upcoming lookahead
                # (slot i+1) or this slot's attn@V reads must be EMITTED
                # before those reads, or the tile deps are missed entirely.
                j = 0
                while j < len(q):
                    if q[j][1] is not None and q[j][1] <= i + 1:
                        q.pop(j)[2]()
                    else:
                        j += 1
                # lookahead scores for the next slot
                sc_nxt = scores(*slots[i + 1]) if i + 1 < len(slots) else None
                at = attnp.tile([128, 1024], bf16, tag="at")
                nc.scalar.activation(
                    at[:], sc_cur[:], AF.Exp,
                    bias=mask_sb[:, kt:kt + 1], scale=0.125,
                )
                # fill the PE exp-wait bubble with independent work
                # (scan past gated units so a blocked head doesn't starve
                # eligible work behind it)
                popped = 0
                if hp == 0:
                    max_pop = 8 if sq == 0 else 3
                elif hp == 1:
                    max_pop = 2
                else:
                    max_pop = 2
                j = 0
                while j < len(q) and popped < max_pop:
                    if q[j][0] <= sq:
                        q.pop(j)[2]()
                        popped += 1
                    else:
                        j += 1
                cb_a, cb_b = cb_cur
                nc.tensor.matmul(
                    cb_a[:],
                    lhsT=v_sb[kt][:, 2 * hp, :],
                    rhs=at[:, 0:512],
                    start=(kt == 0), stop=(kt == SKT - 1),
                )
                nc.tensor.matmul(
                    cb_b[:],
                    lhsT=v_sb[kt][:, 2 * hp + 1, :],
                    rhs=at[:, 512:1024],
                    start=(kt == 0), stop=(kt == SKT - 1),
                )
                sc_cur = sc_nxt
                if kt == SKT - 1:
                    # normalize: comb rows 0..63 / denom(row 64).
                    # Two quick copies free both PSUM banks before the
                    # slow recip/broadcast chains run.
                    cbs_list = []
                    for cb in (cb_a, cb_b):
                        cbs = smallp.tile([65, 512], fp32, tag="cbs", name="cbs")
                        nc.vector.tensor_copy(cbs[:], cb[:])
                        cbs_list.append(cbs)
                    for half_b, cbs in enumerate(cbs_list):
                        rc0 = smallp.tile([1, 512], fp32, tag="rc0")
                        nc.sync.dma_start(rc0[:], cbs[64:65, :])
                        rc1 = smallp.tile([1, 512], fp32, tag="rc1")
                        # approx recip is partition-0 only on HW
                        nc.vector.reciprocal_approx_fast(rc1[:], rc0[:])
                        bc = smallp.tile([64, 512], fp32, tag="bc")
                        nc.gpsimd.partition_broadcast(bc[:], rc1[:])
                        if half_b == 0:
                            nc.vector.tensor_mul(
                                combt_s[hp][0:64, sq * QW:(sq + 1) * QW],
                                cbs[0:64, :], bc[:],
                            )
                        else:
                            tmpb = smallp.tile([64, 512], bf16, tag="tmpb")
                            nc.vector.tensor_mul(tmpb[:], cbs[0:64, :], bc[:])
                            nc.gpsimd.dma_start(
                                combt_s[hp][64:128, sq * QW:(sq + 1) * QW],
                                tmpb[:])

            # ---- tail: leftovers + out-proj of the last query tile ----
            for hp in range(NHP):
                for _, _, u in inject_q[hp]:
                    u()
                inject_q[hp] = []
            for st in range(4 * (SQT - 1), 4 * SQT):
                for half in range(2):
                    st8 = {}
                    for hpc in range(NHP):
                        emit_outproj_unit(st, half, hpc, st8)

    nc.compile()
    return nc


def _get_compiled():
    global _COMPILED
    if _COMPILED is None:
        _COMPILED = _build()
    return _COMPILED


def _prep_core_inputs(x, mask, Wq, bq, Wk, bk, Wv, bv, Wo, core):
    b, hg = core // 2, core % 2
    lo, hi = hg * NHL * HS, (hg + 1) * NHL * HS
    bf = ml_dtypes.bfloat16
    return {
        "xt": np.ascontiguousarray(x[b].T).astype(bf),
        "wq": np.ascontiguousarray(Wq[:, lo:hi]).astype(bf),
        "wk": np.ascontiguousarray(Wk[:, lo:hi]).astype(bf),
        "wv": np.ascontiguousarray(Wv[:, lo:hi]).astype(bf),
        "wo": np.ascontiguousarray(Wo[lo:hi, :]).astype(bf),
        "bq": np.ascontiguousarray(bq[lo:hi].reshape(NHP, 128).T).astype(np.float32),
        "bk": np.ascontiguousarray(bk[lo:hi].reshape(NHP, 128).T).astype(np.float32),
        "bv": np.tile(bv[lo:hi][None, :], (128, 1)).astype(np.float32),
        "mask": np.ascontiguousarray(
            mask[b, 0, 0].reshape(SKT, 128).T).astype(np.float32),
    }


def kernel(x, additive_attention_mask, Wq, bq, Wk, bk, Wv, bv, Wo, bo):
    from concourse import bass2jax

    x = np.asarray(x, dtype=np.float32)
    mask = np.asarray(additive_attention_mask, dtype=np.float32)
    args = [np.asarray(a, dtype=np.float32) for a in (Wq, bq, Wk, bk, Wv, bv, Wo)]
    Wq, bq, Wk, bk, Wv, bv, Wo = args
    bo = np.asarray(bo, dtype=np.float32)

    nc = _get_compiled()
    in_maps = [
        _prep_core_inputs(x, mask, Wq, bq, Wk, bk, Wv, bv, Wo, c)
        for c in range(N_CORES)
    ]
    results = bass2jax.run_bass_via_pjrt(nc, in_maps, n_cores=N_CORES)

    out = np.empty((B, S, H), dtype=np.float32)
    for b in range(B):
        out[b] = results[2 * b]["out"] + results[2 * b + 1]["out"] + bo
    return out


# revision 18
# speedup vs baseline: 1.0161x; 1.0161x over previous
"""BERT self-attention (B=4, S=2048, H=768, 12 heads x 64) on 8 trn2 cores.

Sharding: core c = batch (c//2) x head-half (c%2, 6 heads each).
Each core computes Q/K/V projections for its 6 heads, attention, and a
partial output projection (its heads' slice of Wo). Host sums the two
partials per batch and adds bo.

On-device layout (per core):
  xT   [768, 2048]  bf16  (DMA-transposed x)
  QT/KT per head-pair [128=2x64, 2048] bf16   (head-dim on partitions)
  V    16 tiles [128 keys, 6 heads x 65] bf16 (col 64 of each head = 1.0)
  scores^T [128 keys, 2x512 q] fp32 PSUM (two heads packed via row tiling)
  exp on ScalarE (scale=1/8, bias=mask column), out bf16
  attn@V -> comb [65, 512] PSUM; row 64 = softmax denominator
  combT stacked per head pair [128 = 2x64 dims, 2048] bf16 (head B half
  placed via a small SBUF->SBUF DMA hop), scaled by 1/denom
  out-proj: 3 x K=128 chunks accumulated in PSUM, DVE-drained, DMA out

The attention loop is PE-bound overall; ACT(exp) is a close second.
DMAs are ordered so the first scores (wk, wq, x piece 0) land first,
pulling the first exp to ~8us; V projection and leftover Q/K chains are
injected into the exp-wait bubbles of the early slots instead of
running in a DMA-starved preamble.
"""

import numpy as np
import ml_dtypes

B, S, H = 4, 2048, 768
NH, HS = 12, 64
NHL = 6              # heads per core
NHP = 3              # head pairs per core
HCHUNKS = 6          # 768 / 128 contraction chunks
SKT = 16             # key tiles of 128
SQT = 4              # query tiles of 512
QW = 512             # query tile width
N_CORES = 8

_COMPILED = None


def _build():
    import concourse.bass as bass
    import concourse.mybir as mybir
    import concourse.tile as tile
    from concourse import bacc

    fp32 = mybir.dt.float32
    bf16 = mybir.dt.bfloat16
    AF = mybir.ActivationFunctionType

    nc = bacc.Bacc("TRN2", target_bir_lowering=False, debug=False)

    xt_d = nc.dram_tensor("xt", [H, S], bf16, kind="ExternalInput").ap()
    wq_d = nc.dram_tensor("wq", [H, NHL * HS], bf16, kind="ExternalInput").ap()
    wk_d = nc.dram_tensor("wk", [H, NHL * HS], bf16, kind="ExternalInput").ap()
    wv_d = nc.dram_tensor("wv", [H, NHL * HS], bf16, kind="ExternalInput").ap()
    wo_d = nc.dram_tensor("wo", [NHL * HS, H], bf16, kind="ExternalInput").ap()
    bq_d = nc.dram_tensor("bq", [128, NHP], fp32, kind="ExternalInput").ap()
    bk_d = nc.dram_tensor("bk", [128, NHP], fp32, kind="ExternalInput").ap()
    bv_d = nc.dram_tensor("bv", [128, NHL * HS], fp32, kind="ExternalInput").ap()
    mask_d = nc.dram_tensor("mask", [128, SKT], fp32, kind="ExternalInput").ap()
    out_d = nc.dram_tensor("out", [S, H], fp32, kind="ExternalOutput").ap()

    with tile.TileContext(nc) as tc:
        with (
            tc.tile_pool(name="const", bufs=1) as const,
            tc.tile_pool(name="xt", bufs=1) as xtp,
            tc.tile_pool(name="vsb", bufs=1) as vsb,
            tc.tile_pool(name="qkt", bufs=2) as qkt,
            tc.tile_pool(name="combt", bufs=1) as combtp,
            tc.tile_pool(name="attn", bufs=5) as attnp,
            tc.tile_pool(name="small", bufs=4) as smallp,
            tc.tile_pool(name="outsb", bufs=4) as outsb,
            tc.tile_pool(name="ps_sc", bufs=2, space="PSUM") as ps_sc,
            tc.tile_pool(name="ps_cb", bufs=2, space="PSUM") as ps_cb,
            tc.tile_pool(name="ps_pj", bufs=2, space="PSUM") as ps_pj,
        ):
            # ---- tiles ----
            xt = [[None] * SQT for _ in range(HCHUNKS)]
            for piece in range(SQT):
                for c in range(HCHUNKS):
                    t = xtp.tile([128, QW], bf16, tag=f"xt{c}_{piece}",
                                 name=f"xt{c}_{piece}")
                    xt[c][piece] = t
            wv_sb, wq_sb, wk_sb = [], [], []
            for c in range(HCHUNKS):
                wv_sb.append(const.tile([128, NHL * HS], bf16, tag=f"wv{c}", name=f"wv{c}"))
                wq_sb.append(const.tile([128, NHL * HS], bf16, tag=f"wq{c}", name=f"wq{c}"))
                wk_sb.append(const.tile([128, NHL * HS], bf16, tag=f"wk{c}", name=f"wk{c}"))
            bq_sb = const.tile([128, NHP], fp32, tag="bq")
            bk_sb = const.tile([128, NHP], fp32, tag="bk")
            bv_sb = const.tile([128, NHL * HS], fp32, tag="bv")
            mask_sb = const.tile([128, SKT], fp32, tag="mask")
            wo_sb = [const.tile([128, H], bf16, tag=f"wo{hp}", name=f"wo{hp}")
                     for hp in range(NHP)]
            # head-5 rows of Wo staged at partition base 0 for the tail
            # out-proj (pairs with the last normalize's tmpb, skipping the
            # stacking hop on the critical path)
            wo_hp2b = const.tile([64, H], bf16, tag="wo2b", name="wo2b")
            ones_f = const.tile([1, 64], fp32, tag="ones")
            nc.vector.memset(ones_f[:], 1.0)

            # ---- DMA emission: first-exp dependencies first.
            # sync: x pieces (+mask); scalar: all weights in consumption
            # order (wk/wq gate the first scores, wv the first attn@V,
            # wo only the hp2 phase). gpsimd queue is kept for runtime
            # SBUF->SBUF hops.
            for c in range(HCHUNKS):
                nc.sync.dma_start(xt[c][0][:], xt_d[c * 128:(c + 1) * 128, 0:QW])
            for c in range(HCHUNKS):
                nc.scalar.dma_start(wk_sb[c][:], wk_d[c * 128:(c + 1) * 128, :])
            for c in range(HCHUNKS):
                nc.gpsimd.dma_start(wq_sb[c][:], wq_d[c * 128:(c + 1) * 128, :])
            nc.scalar.dma_start(bk_sb[:], bk_d[:])
            nc.gpsimd.dma_start(bq_sb[:], bq_d[:])
            nc.sync.dma_start(mask_sb[:], mask_d[:])
            for c in range(HCHUNKS):
                nc.scalar.dma_start(wv_sb[c][:], wv_d[c * 128:(c + 1) * 128, :])
            nc.scalar.dma_start(bv_sb[:], bv_d[:])
            for c in range(HCHUNKS):
                nc.sync.dma_start(xt[c][1][:], xt_d[c * 128:(c + 1) * 128, QW:2 * QW])
            for c in range(HCHUNKS):
                nc.sync.dma_start(xt[c][2][:], xt_d[c * 128:(c + 1) * 128, 2 * QW:3 * QW])
            for hp in range(NHP):
                nc.scalar.dma_start(wo_sb[hp][:], wo_d[hp * 128:(hp + 1) * 128, :])
            nc.scalar.dma_start(wo_hp2b[:], wo_d[5 * 64:6 * 64, :])
            for c in range(HCHUNKS):
                nc.sync.dma_start(xt[c][3][:], xt_d[c * 128:(c + 1) * 128, 3 * QW:4 * QW])

            # ---- V projection: V[s, h*65+d], col h*65+64 = 1.0.
            # Emitted as per-kt unit chains, all injected into the
            # attention loop's exp-wait bubbles. ----
            v_sb = [vsb.tile([128, NHL, HS + 1], bf16, tag=f"v{kt}", name=f"v{kt}")
                    for kt in range(SKT)]

            def v_units(kt):
                vt = v_sb[kt]
                st8 = {}

                def unit(c, st8=st8):
                    if c == 0:
                        st8["ps"] = ps_pj.tile([128, 512], fp32, tag="pj",
                                               name="psv")
                    nc.tensor.matmul(
                        st8["ps"][:, :NHL * HS],
                        lhsT=xt[c][kt // 4][:, (kt % 4) * 128:(kt % 4 + 1) * 128],
                        rhs=wv_sb[c][:],
                        start=(c == 0),
                        stop=(c == HCHUNKS - 1),
                    )
                    if c == HCHUNKS - 1:
                        nc.vector.tensor_add(
                            vt[:, :, 0:HS],
                            st8["ps"][:, :NHL * HS].rearrange(
                                "p (h d) -> p h d", h=NHL),
                            bv_sb[:].rearrange("p (h d) -> p h d", h=NHL),
                        )
                        nc.vector.memset(vt[:, :, HS:HS + 1], 1.0)

                return [lambda c=c: unit(c) for c in range(HCHUNKS)]

            # combT stacked per head pair: rows 0-63 = head 2hp,
            # rows 64-127 = head 2hp+1 (written via DMA hop).
            combt_s = [combtp.tile([128, S], bf16, tag=f"cts{hp}", name=f"cts{hp}")
                       for hp in range(NHP)]

            def emit_qkt(hp):
                """Q^T/K^T projection for head pair hp; returns (qt, kt, units).
                Each unit emits one matmul (plus bias-add drain on the last)."""
                qt_t = qkt.tile([128, S], bf16, tag="qt", name=f"qt{hp}")
                kt_t = qkt.tile([128, S], bf16, tag="kt", name=f"kt{hp}")
                units = []
                chains = {}
                for kind, dst, w_sb, b_sb in (("kt", kt_t, wk_sb, bk_sb),
                                              ("qt", qt_t, wq_sb, bq_sb)):
                    for sq in range(SQT):
                        st8 = {}

                        def unit(c, dst=dst, w_sb=w_sb, b_sb=b_sb, sq=sq, st8=st8):
                            if c == 0:
                                st8["ps"] = ps_pj.tile(
                                    [128, 512], fp32, tag="pj", name="psq")
                            nc.tensor.matmul(
                                st8["ps"][:],
                                lhsT=w_sb[c][:, hp * 128:(hp + 1) * 128],
                                rhs=xt[c][sq][:],
                                start=(c == 0),
                                stop=(c == HCHUNKS - 1),
                            )
                            if c == HCHUNKS - 1:
                                nc.vector.tensor_scalar_add(
                                    dst[:, sq * QW:(sq + 1) * QW], st8["ps"][:],
                                    b_sb[:, hp:hp + 1],
                                )

                        chain = [lambda c=c, u=unit: u(c)
                                 for c in range(HCHUNKS)]
                        chains[(kind, sq)] = chain
                        units.extend(chain)
                return qt_t, kt_t, units, chains

            def emit_outproj_unit(st, half, hpc, st8):
                """One of three K=128 matmuls of the out-proj psum chain
                (st, half); hpc = head-pair chunk. Last chunk drains to
                SBUF and streams the result out."""
                if hpc == 0:
                    st8["ps"] = ps_pj.tile([128, 512], fp32, tag="pj", name="pso")
                nc.tensor.matmul(
                    st8["ps"][:, 0:384],
                    lhsT=combt_s[hpc][:, st * 128:(st + 1) * 128],
                    rhs=wo_sb[hpc][:, half * 384:(half + 1) * 384],
                    start=(hpc == 0), stop=(hpc == NHP - 1),
                )
                if hpc == NHP - 1:
                    ob = outsb.tile([128, 384], fp32, tag="ob")
                    nc.vector.tensor_copy(ob[:], st8["ps"][:, 0:384])
                    nc.sync.dma_start(
                        out_d[st * 128:(st + 1) * 128,
                              half * 384:(half + 1) * 384], ob[:])

            def outproj_units(sqs, min_sq):
                """(min_sq, deadline, unit) out-proj chains for the s-tiles
                inside query tiles `sqs` (needs all 3 head pairs' combT)."""
                units = []
                for sq in sqs:
                    for st in range(4 * sq, 4 * (sq + 1)):
                        for half in range(2):
                            st8 = {}
                            for hpc in range(NHP):
                                units.append((min_sq, None,
                                              lambda st=st, half=half,
                                              hpc=hpc, st8=st8:
                                              emit_outproj_unit(st, half, hpc, st8)))
                return units

            # ---- minimal preamble: only what the first two scores need ----
            qkts = [emit_qkt(0)]
            ch0 = qkts[0][3]
            for u in ch0[("kt", 0)] + ch0[("qt", 0)] + ch0[("kt", 1)]:
                u()

            # per-hp injection queues: (min_sq, deadline, emit_fn).
            # deadline = global slot index by whose lookahead-emission the
            # unit must have been EMITTED (its output is read by that
            # slot's scores lookahead or attn@V); None = no deadline.
            inject_q = {0: [], 1: [], 2: []}
            qkts.append(emit_qkt(1))
            q1ch = qkts[1][3]
            q1_main = [u for key in (("kt", 0), ("kt", 1), ("kt", 2), ("kt", 3),
                                     ("qt", 0), ("qt", 1))
                       for u in q1ch[key]]
            inject_q[0] = (
                [(0, kt, u) for kt in range(0, 2) for u in v_units(kt)]
                + [(0, 8, u) for u in ch0[("kt", 2)]]
                + [(0, kt, u) for kt in range(2, 4) for u in v_units(kt)]
                + [(0, 12, u) for u in ch0[("kt", 3)]]
                + [(0, kt, u) for kt in range(4, 6) for u in v_units(kt)]
                + [(0, 16, u) for u in ch0[("qt", 1)]]
                + [(0, kt, u) for kt in range(6, SKT) for u in v_units(kt)]
                + [(1, 32, u) for u in ch0[("qt", 2)]]
                + [(2, 48, u) for u in ch0[("qt", 3)]]
                + [(0, 64, u) for u in q1_main])

            slots = [(hp, sq, kt) for hp in range(NHP) for sq in range(SQT)
                     for kt in range(SKT)]

            def scores(hp, sq, kt):
                qt_t, kt_t = qkts[hp][0], qkts[hp][1]
                sc = ps_sc.tile([128, 1024], fp32, tag="sc", name="sc")
                nc.tensor.matmul(
                    sc[:, 0:512],
                    lhsT=kt_t[0:64, kt * 128:(kt + 1) * 128],
                    rhs=qt_t[0:64, sq * QW:(sq + 1) * QW],
                    start=True, stop=True,
                )
                nc.tensor.matmul(
                    sc[:, 512:1024],
                    lhsT=kt_t[64:128, kt * 128:(kt + 1) * 128],
                    rhs=qt_t[64:128, sq * QW:(sq + 1) * QW],
                    start=True, stop=True,
                )
                return sc

            cb_cur = None
            tail_tmpb = []
            sc_cur = scores(*slots[0])
            for i, (hp, sq, kt) in enumerate(slots):
                if kt == 0:
                    if sq == 0 and hp > 0:
                        # drain any leftover injected work of the previous hp
                        for _, _, u in inject_q[hp - 1]:
                            u()
                        inject_q[hp - 1] = []
                    # build hp-level injection queues lazily at hp start
                    if sq == 0 and hp == 1:
                        qkts.append(emit_qkt(2))
                        inject_q[1] = (
                            [(0, 96, u) for u in q1ch[("qt", 2)]]
                            + [(0, 112, u) for u in q1ch[("qt", 3)]]
                            + [(0, 128, u) for u in qkts[2][2]])
                    if sq == 0 and hp == 2:
                        inject_q[2] = (outproj_units([0], min_sq=1)
                                       + outproj_units([1], min_sq=2)
                                       + outproj_units([2], min_sq=3))
                    cb_a = ps_cb.tile([65, 512], fp32, tag="cb", name="cba")
                    cb_b = ps_cb.tile([65, 512], fp32, tag="cb", name="cbb")
                    cb_cur = (cb_a, cb_b)
                q = inject_q[hp]
                # forced pops: units whose output the upcoming lookahead
                # (slot i+1) or this slot's attn@V reads must be EMITTED
                # before those reads, or the tile deps are missed entirely.
                j = 0
                while j < len(q):
                    if q[j][1] is not None and q[j][1] <= i + 1:
                        q.pop(j)[2]()
                    else:
                        j += 1
                # lookahead scores for the next slot
                sc_nxt = scores(*slots[i + 1]) if i + 1 < len(slots) else None
                at = attnp.tile([128, 1024], bf16, tag="at")
                nc.scalar.activation(
                    at[:], sc_cur[:], AF.Exp,
                    bias=mask_sb[:, kt:kt + 1], scale=0.125,
                )
                # fill the PE exp-wait bubble with independent work
                # (scan past gated units so a blocked head doesn't starve
                # eligible work behind it)
                popped = 0
                if hp == 0:
                    max_pop = 7 if sq == 0 else 2
                elif hp == 1:
                    max_pop = 2
                else:
                    max_pop = 2
                j = 0
                while j < len(q) and popped < max_pop:
                    if q[j][0] <= sq:
                        q.pop(j)[2]()
                        popped += 1
                    else:
                        j += 1
                cb_a, cb_b = cb_cur
                nc.tensor.matmul(
                    cb_a[:],
                    lhsT=v_sb[kt][:, 2 * hp, :],
                    rhs=at[:, 0:512],
                    start=(kt == 0), stop=(kt == SKT - 1),
                )
                nc.tensor.matmul(
                    cb_b[:],
                    lhsT=v_sb[kt][:, 2 * hp + 1, :],
                    rhs=at[:, 512:1024],
                    start=(kt == 0), stop=(kt == SKT - 1),
                )
                sc_cur = sc_nxt
                if kt == SKT - 1:
                    # normalize: comb rows 0..63 / denom(row 64).
                    # Two quick copies free both PSUM banks before the
                    # slow recip/broadcast chains run.
                    last = (hp == NHP - 1 and sq == SQT - 1)
                    cbs_list = []
                    for cb in (cb_a, cb_b):
                        cbs = smallp.tile([65, 512], fp32, tag="cbs", name="cbs")
                        nc.vector.tensor_copy(cbs[:], cb[:])
                        cbs_list.append(cbs)
                    for half_b, cbs in enumerate(cbs_list):
                        rc0 = smallp.tile([1, 512], fp32, tag="rc0")
                        nc.sync.dma_start(rc0[:], cbs[64:65, :])
                        rc1 = smallp.tile([1, 512], fp32, tag="rc1")
                        # approx recip is partition-0 only on HW
                        nc.vector.reciprocal_approx_fast(rc1[:], rc0[:])
                        if last:
                            # tail: broadcast via a PE ones-matmul (PE is
                            # idle here; skips the slow gpsimd broadcast)
                            bcp = ps_pj.tile([128, 512], fp32, tag="pj",
                                             name="bcp")
                            nc.tensor.matmul(bcp[0:64, :], lhsT=ones_f[:],
                                             rhs=rc1[:], start=True, stop=True)
                            bc_ap = bcp[0:64, :]
                        else:
                            bc = smallp.tile([64, 512], fp32, tag="bc")
                            nc.gpsimd.partition_broadcast(bc[:], rc1[:])
                            bc_ap = bc[:]
                        if half_b == 0:
                            nc.vector.tensor_mul(
                                combt_s[hp][0:64, sq * QW:(sq + 1) * QW],
                                cbs[0:64, :], bc_ap,
                            )
                        else:
                            tmpb = smallp.tile([64, 512], bf16, tag="tmpb")
                            nc.vector.tensor_mul(tmpb[:], cbs[0:64, :], bc_ap)
                            if last:
                                tail_tmpb.append(tmpb)
                            else:
                                nc.gpsimd.dma_start(
                                    combt_s[hp][64:128, sq * QW:(sq + 1) * QW],
                                    tmpb[:])

            # ---- tail: leftovers + out-proj of the last query tile.
            # The hp2 slice of the last tile skips the stacking hop: head A
            # reads combt_s[2] rows 0-63, head B reads the normalize temp
            # directly against a base-0 copy of Wo's head-5 rows. ----
            for hp in range(NHP):
                for _, _, u in inject_q[hp]:
                    u()
                inject_q[hp] = []
            tmpb3 = tail_tmpb[0]
            for st in range(4 * (SQT - 1), 4 * SQT):
                for half in range(2):
                    ps = ps_pj.tile([128, 512], fp32, tag="pj", name="pso")
                    for hpc in range(2):
                        nc.tensor.matmul(
                            ps[:, 0:384],
                            lhsT=combt_s[hpc][:, st * 128:(st + 1) * 128],
                            rhs=wo_sb[hpc][:, half * 384:(half + 1) * 384],
                            start=(hpc == 0), stop=False,
                        )
                    nc.tensor.matmul(
                        ps[:, 0:384],
                        lhsT=combt_s[2][0:64, st * 128:(st + 1) * 128],
                        rhs=wo_sb[2][0:64, half * 384:(half + 1) * 384],
                        start=False, stop=False,
                    )
                    nc.tensor.matmul(
                        ps[:, 0:384],
                        lhsT=tmpb3[:, (st - 12) * 128:(st - 11) * 128],
                        rhs=wo_hp2b[:, half * 384:(half + 1) * 384],
                        start=False, stop=True,
                    )
                    ob = outsb.tile([128, 384], fp32, tag="ob")
                    nc.vector.tensor_copy(ob[:], ps[:, 0:384])
                    nc.sync.dma_start(
                        out_d[st * 128:(st + 1) * 128,
                              half * 384:(half + 1) * 384], ob[:])

    nc.compile()
    return nc


def _get_compiled():
    global _COMPILED
    if _COMPILED is None:
        _COMPILED = _build()
    return _COMPILED


def _prep_core_inputs(x, mask, Wq, bq, Wk, bk, Wv, bv, Wo, core):
    b, hg = core // 2, core % 2
    lo, hi = hg * NHL * HS, (hg + 1) * NHL * HS
    bf = ml_dtypes.bfloat16
    return {
        "xt": np.ascontiguousarray(x[b].T).astype(bf),
        "wq": np.ascontiguousarray(Wq[:, lo:hi]).astype(bf),
        "wk": np.ascontiguousarray(Wk[:, lo:hi]).astype(bf),
        "wv": np.ascontiguousarray(Wv[:, lo:hi]).astype(bf),
        "wo": np.ascontiguousarray(Wo[lo:hi, :]).astype(bf),
        "bq": np.ascontiguousarray(bq[lo:hi].reshape(NHP, 128).T).astype(np.float32),
        "bk": np.ascontiguousarray(bk[lo:hi].reshape(NHP, 128).T).astype(np.float32),
        "bv": np.tile(bv[lo:hi][None, :], (128, 1)).astype(np.float32),
        "mask": np.ascontiguousarray(
            mask[b, 0, 0].reshape(SKT, 128).T).astype(np.float32),
    }


def kernel(x, additive_attention_mask, Wq, bq, Wk, bk, Wv, bv, Wo, bo):
    from concourse import bass2jax

    x = np.asarray(x, dtype=np.float32)
    mask = np.asarray(additive_attention_mask, dtype=np.float32)
    args = [np.asarray(a, dtype=np.float32) for a in (Wq, bq, Wk, bk, Wv, bv, Wo)]
    Wq, bq, Wk, bk, Wv, bv, Wo = args
    bo = np.asarray(bo, dtype=np.float32)

    nc = _get_compiled()
    in_maps = [
        _prep_core_inputs(x, mask, Wq, bq, Wk, bk, Wv, bv, Wo, c)
        for c in range(N_CORES)
    ]
    results = bass2jax.run_bass_via_pjrt(nc, in_maps, n_cores=N_CORES)

    out = np.empty((B, S, H), dtype=np.float32)
    for b in range(B):
        out[b] = results[2 * b]["out"] + results[2 * b + 1]["out"] + bo
    return out


# revision 21
# speedup vs baseline: 1.0172x; 1.0011x over previous
"""BERT self-attention (B=4, S=2048, H=768, 12 heads x 64) on 8 trn2 cores.

Sharding: core c = batch (c//2) x head-half (c%2, 6 heads each).
Each core computes Q/K/V projections for its 6 heads, attention, and a
partial output projection (its heads' slice of Wo). Host sums the two
partials per batch and adds bo.

On-device layout (per core):
  xT   [768, 2048]  bf16  (DMA-transposed x)
  QT/KT per head-pair [128=2x64, 2048] bf16   (head-dim on partitions)
  V    16 tiles [128 keys, 6 heads x 65] bf16 (col 64 of each head = 1.0)
  scores^T [128 keys, 2x512 q] fp32 PSUM (two heads packed via row tiling)
  exp on ScalarE (scale=1/8, bias=mask column), out bf16
  attn@V -> comb [65, 512] PSUM; row 64 = softmax denominator
  combT stacked per head pair [128 = 2x64 dims, 2048] bf16 (head B half
  placed via a small SBUF->SBUF DMA hop), scaled by 1/denom
  out-proj: 3 x K=128 chunks accumulated in PSUM, DVE-drained, DMA out

The attention loop is PE-bound overall; ACT(exp) is a close second.
DMAs are ordered so the first scores (wk, wq, x piece 0) land first,
pulling the first exp to ~8us; V projection and leftover Q/K chains are
injected into the exp-wait bubbles of the early slots instead of
running in a DMA-starved preamble.
"""

import numpy as np
import ml_dtypes

B, S, H = 4, 2048, 768
NH, HS = 12, 64
NHL = 6              # heads per core
NHP = 3              # head pairs per core
HCHUNKS = 6          # 768 / 128 contraction chunks
SKT = 16             # key tiles of 128
SQT = 4              # query tiles of 512
QW = 512             # query tile width
N_CORES = 8

_COMPILED = None


def _build():
    import concourse.bass as bass
    import concourse.mybir as mybir
    import concourse.tile as tile
    from concourse import bacc

    fp32 = mybir.dt.float32
    bf16 = mybir.dt.bfloat16
    AF = mybir.ActivationFunctionType

    nc = bacc.Bacc("TRN2", target_bir_lowering=False, debug=False)

    xt_d = nc.dram_tensor("xt", [H, S], bf16, kind="ExternalInput").ap()
    wq_d = nc.dram_tensor("wq", [H, NHL * HS], bf16, kind="ExternalInput").ap()
    wk_d = nc.dram_tensor("wk", [H, NHL * HS], bf16, kind="ExternalInput").ap()
    wv_d = nc.dram_tensor("wv", [H, NHL * HS], bf16, kind="ExternalInput").ap()
    wo_d = nc.dram_tensor("wo", [NHL * HS, H], bf16, kind="ExternalInput").ap()
    bq_d = nc.dram_tensor("bq", [128, NHP], fp32, kind="ExternalInput").ap()
    bk_d = nc.dram_tensor("bk", [128, NHP], fp32, kind="ExternalInput").ap()
    bv_d = nc.dram_tensor("bv", [128, NHL * HS], fp32, kind="ExternalInput").ap()
    mask_d = nc.dram_tensor("mask", [128, SKT], fp32, kind="ExternalInput").ap()
    out_d = nc.dram_tensor("out", [S, H], fp32, kind="ExternalOutput").ap()

    with tile.TileContext(nc) as tc:
        with (
            tc.tile_pool(name="const", bufs=1) as const,
            tc.tile_pool(name="xt", bufs=1) as xtp,
            tc.tile_pool(name="vsb", bufs=1) as vsb,
            tc.tile_pool(name="qkt", bufs=2) as qkt,
            tc.tile_pool(name="combt", bufs=1) as combtp,
            tc.tile_pool(name="attn", bufs=5) as attnp,
            tc.tile_pool(name="small", bufs=4) as smallp,
            tc.tile_pool(name="outsb", bufs=4) as outsb,
            tc.tile_pool(name="ps_sc", bufs=2, space="PSUM") as ps_sc,
            tc.tile_pool(name="ps_cb", bufs=2, space="PSUM") as ps_cb,
            tc.tile_pool(name="ps_pj", bufs=2, space="PSUM") as ps_pj,
        ):
            # ---- tiles ----
            xt = [[None] * SQT for _ in range(HCHUNKS)]
            for piece in range(SQT):
                for c in range(HCHUNKS):
                    t = xtp.tile([128, QW], bf16, tag=f"xt{c}_{piece}",
                                 name=f"xt{c}_{piece}")
                    xt[c][piece] = t
            wv_sb, wq_sb, wk_sb = [], [], []
            for c in range(HCHUNKS):
                wv_sb.append(const.tile([128, NHL * HS], bf16, tag=f"wv{c}", name=f"wv{c}"))
                wq_sb.append(const.tile([128, NHL * HS], bf16, tag=f"wq{c}", name=f"wq{c}"))
                wk_sb.append(const.tile([128, NHL * HS], bf16, tag=f"wk{c}", name=f"wk{c}"))
            bq_sb = const.tile([128, NHP], fp32, tag="bq")
            bk_sb = const.tile([128, NHP], fp32, tag="bk")
            bv_sb = const.tile([128, NHL * HS], fp32, tag="bv")
            mask_sb = const.tile([128, SKT], fp32, tag="mask")
            wo_sb = [const.tile([128, H], bf16, tag=f"wo{hp}", name=f"wo{hp}")
                     for hp in range(NHP)]
            # head-5 rows of Wo staged at partition base 0 for the tail
            # out-proj (pairs with the last normalize's tmpb, skipping the
            # stacking hop on the critical path)
            wo_hp2b = const.tile([64, H], bf16, tag="wo2b", name="wo2b")
            ones_f = const.tile([1, 64], fp32, tag="ones")
            nc.vector.memset(ones_f[:], 1.0)

            # ---- DMA emission: first-exp dependencies first.
            # sync: x pieces (+mask); scalar: all weights in consumption
            # order (wk/wq gate the first scores, wv the first attn@V,
            # wo only the hp2 phase). gpsimd queue is kept for runtime
            # SBUF->SBUF hops.
            for c in range(HCHUNKS):
                nc.sync.dma_start(xt[c][0][:], xt_d[c * 128:(c + 1) * 128, 0:QW])
            for c in range(HCHUNKS):
                nc.scalar.dma_start(wk_sb[c][:], wk_d[c * 128:(c + 1) * 128, :])
            for c in range(HCHUNKS):
                nc.gpsimd.dma_start(wq_sb[c][:], wq_d[c * 128:(c + 1) * 128, :])
            nc.scalar.dma_start(bk_sb[:], bk_d[:])
            nc.gpsimd.dma_start(bq_sb[:], bq_d[:])
            nc.sync.dma_start(mask_sb[:], mask_d[:])
            # x piece 1 on the scalar queue right behind wk: the preamble's
            # kt1 chain consumes it and gates the first exp
            for c in range(HCHUNKS):
                nc.scalar.dma_start(xt[c][1][:], xt_d[c * 128:(c + 1) * 128, QW:2 * QW])
            for c in range(HCHUNKS):
                nc.scalar.dma_start(wv_sb[c][:], wv_d[c * 128:(c + 1) * 128, :])
            nc.scalar.dma_start(bv_sb[:], bv_d[:])
            for c in range(HCHUNKS):
                nc.sync.dma_start(xt[c][2][:], xt_d[c * 128:(c + 1) * 128, 2 * QW:3 * QW])
            for hp in range(NHP):
                nc.scalar.dma_start(wo_sb[hp][:], wo_d[hp * 128:(hp + 1) * 128, :])
            nc.scalar.dma_start(wo_hp2b[:], wo_d[5 * 64:6 * 64, :])
            for c in range(HCHUNKS):
                nc.sync.dma_start(xt[c][3][:], xt_d[c * 128:(c + 1) * 128, 3 * QW:4 * QW])

            # ---- V projection: V[s, h*65+d], col h*65+64 = 1.0.
            # Emitted as per-kt unit chains, all injected into the
            # attention loop's exp-wait bubbles. ----
            v_sb = [vsb.tile([128, NHL, HS + 1], bf16, tag=f"v{kt}", name=f"v{kt}")
                    for kt in range(SKT)]

            def v_units(kt):
                vt = v_sb[kt]
                st8 = {}

                def unit(c, st8=st8):
                    if c == 0:
                        st8["ps"] = ps_pj.tile([128, 512], fp32, tag="pj",
                                               name="psv")
                    nc.tensor.matmul(
                        st8["ps"][:, :NHL * HS],
                        lhsT=xt[c][kt // 4][:, (kt % 4) * 128:(kt % 4 + 1) * 128],
                        rhs=wv_sb[c][:],
                        start=(c == 0),
                        stop=(c == HCHUNKS - 1),
                    )
                    if c == HCHUNKS - 1:
                        nc.vector.tensor_add(
                            vt[:, :, 0:HS],
                            st8["ps"][:, :NHL * HS].rearrange(
                                "p (h d) -> p h d", h=NHL),
                            bv_sb[:].rearrange("p (h d) -> p h d", h=NHL),
                        )
                        nc.vector.memset(vt[:, :, HS:HS + 1], 1.0)

                return [lambda c=c: unit(c) for c in range(HCHUNKS)]

            # combT stacked per head pair: rows 0-63 = head 2hp,
            # rows 64-127 = head 2hp+1 (written via DMA hop).
            combt_s = [combtp.tile([128, S], bf16, tag=f"cts{hp}", name=f"cts{hp}")
                       for hp in range(NHP)]

            def emit_qkt(hp):
                """Q^T/K^T projection for head pair hp; returns (qt, kt, units).
                Each unit emits one matmul (plus bias-add drain on the last)."""
                qt_t = qkt.tile([128, S], bf16, tag="qt", name=f"qt{hp}")
                kt_t = qkt.tile([128, S], bf16, tag="kt", name=f"kt{hp}")
                units = []
                chains = {}
                for kind, dst, w_sb, b_sb in (("kt", kt_t, wk_sb, bk_sb),
                                              ("qt", qt_t, wq_sb, bq_sb)):
                    for sq in range(SQT):
                        st8 = {}

                        def unit(c, dst=dst, w_sb=w_sb, b_sb=b_sb, sq=sq, st8=st8):
                            if c == 0:
                                st8["ps"] = ps_pj.tile(
                                    [128, 512], fp32, tag="pj", name="psq")
                            nc.tensor.matmul(
                                st8["ps"][:],
                                lhsT=w_sb[c][:, hp * 128:(hp + 1) * 128],
                                rhs=xt[c][sq][:],
                                start=(c == 0),
                                stop=(c == HCHUNKS - 1),
                            )
                            if c == HCHUNKS - 1:
                                nc.vector.tensor_scalar_add(
                                    dst[:, sq * QW:(sq + 1) * QW], st8["ps"][:],
                                    b_sb[:, hp:hp + 1],
                                )

                        chain = [lambda c=c, u=unit: u(c)
                                 for c in range(HCHUNKS)]
                        chains[(kind, sq)] = chain
                        units.extend(chain)
                return qt_t, kt_t, units, chains

            def emit_outproj_unit(st, half, hpc, st8):
                """One of three K=128 matmuls of the out-proj psum chain
                (st, half); hpc = head-pair chunk. Last chunk drains to
                SBUF and streams the result out."""
                if hpc == 0:
                    st8["ps"] = ps_pj.tile([128, 512], fp32, tag="pj", name="pso")
                nc.tensor.matmul(
                    st8["ps"][:, 0:384],
                    lhsT=combt_s[hpc][:, st * 128:(st + 1) * 128],
                    rhs=wo_sb[hpc][:, half * 384:(half + 1) * 384],
                    start=(hpc == 0), stop=(hpc == NHP - 1),
                )
                if hpc == NHP - 1:
                    ob = outsb.tile([128, 384], fp32, tag="ob")
                    nc.vector.tensor_copy(ob[:], st8["ps"][:, 0:384])
                    nc.sync.dma_start(
                        out_d[st * 128:(st + 1) * 128,
                              half * 384:(half + 1) * 384], ob[:])

            def outproj_units(sqs, min_sq):
                """(min_sq, deadline, unit) out-proj chains for the s-tiles
                inside query tiles `sqs` (needs all 3 head pairs' combT)."""
                units = []
                for sq in sqs:
                    for st in range(4 * sq, 4 * (sq + 1)):
                        for half in range(2):
                            st8 = {}
                            for hpc in range(NHP):
                                units.append((min_sq, None,
                                              lambda st=st, half=half,
                                              hpc=hpc, st8=st8:
                                              emit_outproj_unit(st, half, hpc, st8)))
                return units

            # ---- minimal preamble: only what the first two scores need ----
            qkts = [emit_qkt(0)]
            ch0 = qkts[0][3]
            for u in ch0[("kt", 0)] + ch0[("qt", 0)] + ch0[("kt", 1)]:
                u()

            # per-hp injection queues: (min_sq, deadline, emit_fn).
            # deadline = global slot index by whose lookahead-emission the
            # unit must have been EMITTED (its output is read by that
            # slot's scores lookahead or attn@V); None = no deadline.
            inject_q = {0: [], 1: [], 2: []}
            qkts.append(emit_qkt(1))
            q1ch = qkts[1][3]
            q1_main = [u for key in (("kt", 0), ("kt", 1), ("kt", 2), ("kt", 3),
                                     ("qt", 0), ("qt", 1))
                       for u in q1ch[key]]
            inject_q[0] = (
                [(0, kt, u) for kt in range(0, 2) for u in v_units(kt)]
                + [(0, 8, u) for u in ch0[("kt", 2)]]
                + [(0, kt, u) for kt in range(2, 4) for u in v_units(kt)]
                + [(0, 12, u) for u in ch0[("kt", 3)]]
                + [(0, kt, u) for kt in range(4, 6) for u in v_units(kt)]
                + [(0, 16, u) for u in ch0[("qt", 1)]]
                + [(0, kt, u) for kt in range(6, SKT) for u in v_units(kt)]
                + [(1, 32, u) for u in ch0[("qt", 2)]]
                + [(2, 48, u) for u in ch0[("qt", 3)]]
                + [(0, 64, u) for u in q1_main])

            slots = [(hp, sq, kt) for hp in range(NHP) for sq in range(SQT)
                     for kt in range(SKT)]

            def scores(hp, sq, kt):
                qt_t, kt_t = qkts[hp][0], qkts[hp][1]
                sc = ps_sc.tile([128, 1024], fp32, tag="sc", name="sc")
                nc.tensor.matmul(
                    sc[:, 0:512],
                    lhsT=kt_t[0:64, kt * 128:(kt + 1) * 128],
                    rhs=qt_t[0:64, sq * QW:(sq + 1) * QW],
                    start=True, stop=True,
                )
                nc.tensor.matmul(
                    sc[:, 512:1024],
                    lhsT=kt_t[64:128, kt * 128:(kt + 1) * 128],
                    rhs=qt_t[64:128, sq * QW:(sq + 1) * QW],
                    start=True, stop=True,
                )
                return sc

            cb_cur = None
            tail_tmpb = []
            sc_cur = scores(*slots[0])
            for i, (hp, sq, kt) in enumerate(slots):
                if kt == 0:
                    if sq == 0 and hp > 0:
                        # drain any leftover injected work of the previous hp
                        for _, _, u in inject_q[hp - 1]:
                            u()
                        inject_q[hp - 1] = []
                    # build hp-level injection queues lazily at hp start
                    if sq == 0 and hp == 1:
                        qkts.append(emit_qkt(2))
                        inject_q[1] = (
                            [(0, 96, u) for u in q1ch[("qt", 2)]]
                            + [(0, 112, u) for u in q1ch[("qt", 3)]]
                            + [(0, 128, u) for u in qkts[2][2]])
                    if sq == 0 and hp == 2:
                        inject_q[2] = (outproj_units([0], min_sq=1)
                                       + outproj_units([1], min_sq=2)
                                       + outproj_units([2], min_sq=3))
                    cb_a = ps_cb.tile([65, 512], fp32, tag="cb", name="cba")
                    cb_b = ps_cb.tile([65, 512], fp32, tag="cb", name="cbb")
                    cb_cur = (cb_a, cb_b)
                q = inject_q[hp]
                # forced pops: units whose output the upcoming lookahead
                # (slot i+1) or this slot's attn@V reads must be EMITTED
                # before those reads, or the tile deps are missed entirely.
                j = 0
                while j < len(q):
                    if q[j][1] is not None and q[j][1] <= i + 1:
                        q.pop(j)[2]()
                    else:
                        j += 1
                # lookahead scores for the next slot
                sc_nxt = scores(*slots[i + 1]) if i + 1 < len(slots) else None
                at = attnp.tile([128, 1024], bf16, tag="at")
                nc.scalar.activation(
                    at[:], sc_cur[:], AF.Exp,
                    bias=mask_sb[:, kt:kt + 1], scale=0.125,
                )
                # fill the PE exp-wait bubble with independent work
                # (scan past gated units so a blocked head doesn't starve
                # eligible work behind it)
                popped = 0
                if hp == 0:
                    max_pop = 7 if sq == 0 else 2
                elif hp == 1:
                    max_pop = 2
                else:
                    max_pop = 2
                j = 0
                while j < len(q) and popped < max_pop:
                    if q[j][0] <= sq:
                        q.pop(j)[2]()
                        popped += 1
                    else:
                        j += 1
                cb_a, cb_b = cb_cur
                nc.tensor.matmul(
                    cb_a[:],
                    lhsT=v_sb[kt][:, 2 * hp, :],
                    rhs=at[:, 0:512],
                    start=(kt == 0), stop=(kt == SKT - 1),
                )
                nc.tensor.matmul(
                    cb_b[:],
                    lhsT=v_sb[kt][:, 2 * hp + 1, :],
                    rhs=at[:, 512:1024],
                    start=(kt == 0), stop=(kt == SKT - 1),
                )
                sc_cur = sc_nxt
                if i == len(slots) - 1:
                    # keep the PE active through the serial tail normalize
                    # so the HAM clock gate stays at full rate for the
                    # tail out-proj matmuls
                    for _ in range(8):
                        dmy = ps_sc.tile([128, 1024], fp32, tag="sc",
                                         name="dmy")
                        nc.tensor.matmul(
                            dmy[:, 0:512],
                            lhsT=qkts[2][1][0:64, 0:128],
                            rhs=qkts[2][0][0:64, 0:512],
                            start=True, stop=True,
                        )
                if kt == SKT - 1:
                    # normalize: comb rows 0..63 / denom(row 64).
                    # Two quick copies free both PSUM banks before the
                    # slow recip/broadcast chains run.
                    last = (hp == NHP - 1 and sq == SQT - 1)
                    cbs_list = []
                    for cb in (cb_a, cb_b):
                        cbs = smallp.tile([65, 512], fp32, tag="cbs", name="cbs")
                        nc.vector.tensor_copy(cbs[:], cb[:])
                        cbs_list.append(cbs)
                    for half_b, cbs in enumerate(cbs_list):
                        rc0 = smallp.tile([1, 512], fp32, tag="rc0")
                        nc.sync.dma_start(rc0[:], cbs[64:65, :])
                        rc1 = smallp.tile([1, 512], fp32, tag="rc1")
                        # approx recip is partition-0 only on HW
                        nc.vector.reciprocal_approx_fast(rc1[:], rc0[:])
                        if last:
                            # tail: broadcast via a PE ones-matmul (PE is
                            # idle here; skips the slow gpsimd broadcast)
                            bcp = ps_pj.tile([128, 512], fp32, tag="pj",
                                             name="bcp")
                            nc.tensor.matmul(bcp[0:64, :], lhsT=ones_f[:],
                                             rhs=rc1[:], start=True, stop=True)
                            bc_ap = bcp[0:64, :]
                        else:
                            bc = smallp.tile([64, 512], fp32, tag="bc")
                            nc.gpsimd.partition_broadcast(bc[:], rc1[:])
                            bc_ap = bc[:]
                        if half_b == 0:
                            nc.vector.tensor_mul(
                                combt_s[hp][0:64, sq * QW:(sq + 1) * QW],
                                cbs[0:64, :], bc_ap,
                            )
                        else:
                            tmpb = smallp.tile([64, 512], bf16, tag="tmpb")
                            nc.vector.tensor_mul(tmpb[:], cbs[0:64, :], bc_ap)
                            if last:
                                tail_tmpb.append(tmpb)
                            else:
                                nc.gpsimd.dma_start(
                                    combt_s[hp][64:128, sq * QW:(sq + 1) * QW],
                                    tmpb[:])

            # ---- tail: leftovers + out-proj of the last query tile.
            # The hp2 slice of the last tile skips the stacking hop: head A
            # reads combt_s[2] rows 0-63, head B reads the normalize temp
            # directly against a base-0 copy of Wo's head-5 rows. ----
            for hp in range(NHP):
                for _, _, u in inject_q[hp]:
                    u()
                inject_q[hp] = []
            tmpb3 = tail_tmpb[0]
            chain_i = 0
            for st in range(4 * (SQT - 1), 4 * SQT):
                for half in range(2):
                    # alternate PSUM pools for a 4-deep chain rotation
                    # (ps_sc is idle in the tail)
                    if chain_i % 2 == 0:
                        ps = ps_pj.tile([128, 512], fp32, tag="pj", name="pso")
                    else:
                        ps = ps_sc.tile([128, 1024], fp32, tag="sc", name="pso")
                    chain_i += 1
                    for hpc in range(2):
                        nc.tensor.matmul(
                            ps[:, 0:384],
                            lhsT=combt_s[hpc][:, st * 128:(st + 1) * 128],
                            rhs=wo_sb[hpc][:, half * 384:(half + 1) * 384],
                            start=(hpc == 0), stop=False,
                        )
                    nc.tensor.matmul(
                        ps[:, 0:384],
                        lhsT=combt_s[2][0:64, st * 128:(st + 1) * 128],
                        rhs=wo_sb[2][0:64, half * 384:(half + 1) * 384],
                        start=False, stop=False,
                    )
                    nc.tensor.matmul(
                        ps[:, 0:384],
                        lhsT=tmpb3[:, (st - 12) * 128:(st - 11) * 128],
                        rhs=wo_hp2b[:, half * 384:(half + 1) * 384],
                        start=False, stop=True,
                    )
                    ob = outsb.tile([128, 384], fp32, tag="ob")
                    nc.vector.tensor_copy(ob[:], ps[:, 0:384])
                    nc.sync.dma_start(
                        out_d[st * 128:(st + 1) * 128,
                              half * 384:(half + 1) * 384], ob[:])

    nc.compile()
    return nc


def _get_compiled():
    global _COMPILED
    if _COMPILED is None:
        _COMPILED = _build()
    return _COMPILED


def _prep_core_inputs(x, mask, Wq, bq, Wk, bk, Wv, bv, Wo, core):
    b, hg = core // 2, core % 2
    lo, hi = hg * NHL * HS, (hg + 1) * NHL * HS
    bf = ml_dtypes.bfloat16
    return {
        "xt": np.ascontiguousarray(x[b].T).astype(bf),
        "wq": np.ascontiguousarray(Wq[:, lo:hi]).astype(bf),
        "wk": np.ascontiguousarray(Wk[:, lo:hi]).astype(bf),
        "wv": np.ascontiguousarray(Wv[:, lo:hi]).astype(bf),
        "wo": np.ascontiguousarray(Wo[lo:hi, :]).astype(bf),
        "bq": np.ascontiguousarray(bq[lo:hi].reshape(NHP, 128).T).astype(np.float32),
        "bk": np.ascontiguousarray(bk[lo:hi].reshape(NHP, 128).T).astype(np.float32),
        "bv": np.tile(bv[lo:hi][None, :], (128, 1)).astype(np.float32),
        "mask": np.ascontiguousarray(
            mask[b, 0, 0].reshape(SKT, 128).T).astype(np.float32),
    }


def kernel(x, additive_attention_mask, Wq, bq, Wk, bk, Wv, bv, Wo, bo):
    from concourse import bass2jax

    x = np.asarray(x, dtype=np.float32)
    mask = np.asarray(additive_attention_mask, dtype=np.float32)
    args = [np.asarray(a, dtype=np.float32) for a in (Wq, bq, Wk, bk, Wv, bv, Wo)]
    Wq, bq, Wk, bk, Wv, bv, Wo = args
    bo = np.asarray(bo, dtype=np.float32)

    nc = _get_compiled()
    in_maps = [
        _prep_core_inputs(x, mask, Wq, bq, Wk, bk, Wv, bv, Wo, c)
        for c in range(N_CORES)
    ]
    results = bass2jax.run_bass_via_pjrt(nc, in_maps, n_cores=N_CORES)

    out = np.empty((B, S, H), dtype=np.float32)
    for b in range(B):
        out[b] = results[2 * b]["out"] + results[2 * b + 1]["out"] + bo
    return out


# revision 24
# speedup vs baseline: 1.0367x; 1.0191x over previous
"""BERT self-attention (B=4, S=2048, H=768, 12 heads x 64) on 8 trn2 cores.

Sharding: core c = batch (c//2) x head-half (c%2, 6 heads each).
Each core computes Q/K/V projections for its 6 heads, attention, and a
partial output projection (its heads' slice of Wo). Host sums the two
partials per batch and adds bo.

On-device layout (per core):
  xT   [768, 2048]  bf16  (DMA-transposed x)
  QT/KT per head-pair [128=2x64, 2048] bf16   (head-dim on partitions)
  V    16 tiles [128 keys, 6 heads x 65] bf16 (col 64 of each head = 1.0)
  scores^T [128 keys, 2x512 q] fp32 PSUM (two heads packed via row tiling)
  exp on ScalarE (scale=1/8, bias=mask column), out bf16
  attn@V -> comb [65, 512] PSUM; row 64 = softmax denominator
  combT stacked per head pair [128 = 2x64 dims, 2048] bf16 (head B half
  placed via a small SBUF->SBUF DMA hop), scaled by 1/denom
  out-proj: 3 x K=128 chunks accumulated in PSUM, DVE-drained, DMA out

The attention loop is PE-bound overall; ACT(exp) is a close second.
DMAs are ordered so the first scores (wk, wq, x piece 0) land first,
pulling the first exp to ~8us; V projection and leftover Q/K chains are
injected into the exp-wait bubbles of the early slots instead of
running in a DMA-starved preamble.
"""

import numpy as np
import ml_dtypes

B, S, H = 4, 2048, 768
NH, HS = 12, 64
NHL = 6              # heads per core
NHP = 3              # head pairs per core
HCHUNKS = 6          # 768 / 128 contraction chunks
SKT = 16             # key tiles of 128
SQT = 4              # query tiles of 512
QW = 512             # query tile width
N_CORES = 8

_COMPILED = None


def _build():
    import concourse.bass as bass
    import concourse.mybir as mybir
    import concourse.tile as tile
    from concourse import bacc

    fp32 = mybir.dt.float32
    bf16 = mybir.dt.bfloat16
    AF = mybir.ActivationFunctionType

    nc = bacc.Bacc("TRN2", target_bir_lowering=False, debug=False)

    xt_d = nc.dram_tensor("xt", [H, S], bf16, kind="ExternalInput").ap()
    wq_d = nc.dram_tensor("wq", [H, NHL * HS], bf16, kind="ExternalInput").ap()
    wk_d = nc.dram_tensor("wk", [H, NHL * HS], bf16, kind="ExternalInput").ap()
    wv_d = nc.dram_tensor("wv", [H, NHL * HS], bf16, kind="ExternalInput").ap()
    wo_d = nc.dram_tensor("wo", [NHL * HS, H], bf16, kind="ExternalInput").ap()
    bq_d = nc.dram_tensor("bq", [128, NHP], fp32, kind="ExternalInput").ap()
    bk_d = nc.dram_tensor("bk", [128, NHP], fp32, kind="ExternalInput").ap()
    bv_d = nc.dram_tensor("bv", [128, NHL * HS], fp32, kind="ExternalInput").ap()
    mask_d = nc.dram_tensor("mask", [128, SKT], fp32, kind="ExternalInput").ap()
    out_d = nc.dram_tensor("out", [S, H], fp32, kind="ExternalOutput").ap()

    with tile.TileContext(nc) as tc:
        with (
            tc.tile_pool(name="const", bufs=1) as const,
            tc.tile_pool(name="xt", bufs=1) as xtp,
            tc.tile_pool(name="vsb", bufs=1) as vsb,
            tc.tile_pool(name="qkt", bufs=2) as qkt,
            tc.tile_pool(name="combt", bufs=1) as combtp,
            tc.tile_pool(name="attn", bufs=5) as attnp,
            tc.tile_pool(name="small", bufs=4) as smallp,
            tc.tile_pool(name="outsb", bufs=4) as outsb,
            tc.tile_pool(name="ps_sc", bufs=2, space="PSUM") as ps_sc,
            tc.tile_pool(name="ps_cb", bufs=2, space="PSUM") as ps_cb,
            tc.tile_pool(name="ps_pj", bufs=2, space="PSUM") as ps_pj,
        ):
            # ---- tiles ----
            # x pieces and weights are single wide tiles filled by one
            # strided DMA each (startup DMAs are issue-cost bound; fewer
            # and bigger transfers land the first-exp deps much earlier)
            xt_all = [xtp.tile([128, HCHUNKS, QW], bf16, tag=f"xt{p}",
                               name=f"xt{p}") for p in range(SQT)]
            xt = [[xt_all[p][:, c, :] for p in range(SQT)]
                  for c in range(HCHUNKS)]
            wv_all = const.tile([128, HCHUNKS, NHL * HS], bf16, tag="wv")
            wq_all = const.tile([128, HCHUNKS, NHL * HS], bf16, tag="wq")
            wk_all = const.tile([128, HCHUNKS, NHL * HS], bf16, tag="wk")
            wv_sb = [wv_all[:, c, :] for c in range(HCHUNKS)]
            wq_sb = [wq_all[:, c, :] for c in range(HCHUNKS)]
            wk_sb = [wk_all[:, c, :] for c in range(HCHUNKS)]
            bq_sb = const.tile([128, NHP], fp32, tag="bq")
            bk_sb = const.tile([128, NHP], fp32, tag="bk")
            bv_sb = const.tile([128, NHL * HS], fp32, tag="bv")
            mask_sb = const.tile([128, SKT], fp32, tag="mask")
            wo_all = const.tile([128, NHP, H], bf16, tag="wo")
            wo_sb = [wo_all[:, hp, :] for hp in range(NHP)]
            # head-5 rows of Wo staged at partition base 0 for the tail
            # out-proj (pairs with the last normalize's tmpb, skipping the
            # stacking hop on the critical path)
            wo_hp2b = const.tile([64, H], bf16, tag="wo2b", name="wo2b")
            ones_f = const.tile([1, 64], fp32, tag="ones")
            nc.vector.memset(ones_f[:], 1.0)

            # ---- DMA emission: first-exp dependencies first.
            # sync: x pieces (+mask); scalar: all weights in consumption
            # order (wk/wq gate the first scores, wv the first attn@V,
            # wo only the hp2 phase). gpsimd queue is kept for runtime
            # SBUF->SBUF hops.
            xt_r = xt_d.rearrange("(c p) s -> p c s", c=HCHUNKS)
            nc.sync.dma_start(xt_all[0][:], xt_r[:, :, 0:QW])
            nc.scalar.dma_start(
                wk_all[:], wk_d.rearrange("(c p) w -> p c w", c=HCHUNKS))
            nc.gpsimd.dma_start(
                wq_all[:], wq_d.rearrange("(c p) w -> p c w", c=HCHUNKS))
            nc.scalar.dma_start(bk_sb[:], bk_d[:])
            nc.gpsimd.dma_start(bq_sb[:], bq_d[:])
            nc.sync.dma_start(mask_sb[:], mask_d[:])
            nc.sync.dma_start(xt_all[1][:], xt_r[:, :, QW:2 * QW])
            nc.scalar.dma_start(
                wv_all[:], wv_d.rearrange("(c p) w -> p c w", c=HCHUNKS))
            nc.scalar.dma_start(bv_sb[:], bv_d[:])
            nc.sync.dma_start(xt_all[2][:], xt_r[:, :, 2 * QW:3 * QW])
            nc.scalar.dma_start(
                wo_all[:], wo_d.rearrange("(c p) h -> p c h", c=NHP))
            nc.scalar.dma_start(wo_hp2b[:], wo_d[5 * 64:6 * 64, :])
            nc.sync.dma_start(xt_all[3][:], xt_r[:, :, 3 * QW:4 * QW])

            # ---- V projection: V[s, h*65+d], col h*65+64 = 1.0.
            # Emitted as per-kt unit chains, all injected into the
            # attention loop's exp-wait bubbles. ----
            v_sb = [vsb.tile([128, NHL, HS + 1], bf16, tag=f"v{kt}", name=f"v{kt}")
                    for kt in range(SKT)]

            def v_units(kt):
                vt = v_sb[kt]
                st8 = {}

                def unit(c, st8=st8):
                    if c == 0:
                        st8["ps"] = ps_pj.tile([128, 512], fp32, tag="pj",
                                               name="psv")
                    nc.tensor.matmul(
                        st8["ps"][:, :NHL * HS],
                        lhsT=xt[c][kt // 4][:, (kt % 4) * 128:(kt % 4 + 1) * 128],
                        rhs=wv_sb[c][:],
                        start=(c == 0),
                        stop=(c == HCHUNKS - 1),
                    )
                    if c == HCHUNKS - 1:
                        nc.vector.tensor_add(
                            vt[:, :, 0:HS],
                            st8["ps"][:, :NHL * HS].rearrange(
                                "p (h d) -> p h d", h=NHL),
                            bv_sb[:].rearrange("p (h d) -> p h d", h=NHL),
                        )
                        nc.vector.memset(vt[:, :, HS:HS + 1], 1.0)

                return [lambda c=c: unit(c) for c in range(HCHUNKS)]

            # combT stacked per head pair: rows 0-63 = head 2hp,
            # rows 64-127 = head 2hp+1 (written via DMA hop).
            combt_s = [combtp.tile([128, S], bf16, tag=f"cts{hp}", name=f"cts{hp}")
                       for hp in range(NHP)]

            def emit_qkt(hp):
                """Q^T/K^T projection for head pair hp; returns (qt, kt, units).
                Each unit emits one matmul (plus bias-add drain on the last)."""
                qt_t = qkt.tile([128, S], bf16, tag="qt", name=f"qt{hp}")
                kt_t = qkt.tile([128, S], bf16, tag="kt", name=f"kt{hp}")
                units = []
                chains = {}
                for kind, dst, w_sb, b_sb in (("kt", kt_t, wk_sb, bk_sb),
                                              ("qt", qt_t, wq_sb, bq_sb)):
                    for sq in range(SQT):
                        st8 = {}

                        def unit(c, dst=dst, w_sb=w_sb, b_sb=b_sb, sq=sq, st8=st8):
                            if c == 0:
                                st8["ps"] = ps_pj.tile(
                                    [128, 512], fp32, tag="pj", name="psq")
                            nc.tensor.matmul(
                                st8["ps"][:],
                                lhsT=w_sb[c][:, hp * 128:(hp + 1) * 128],
                                rhs=xt[c][sq][:],
                                start=(c == 0),
                                stop=(c == HCHUNKS - 1),
                            )
                            if c == HCHUNKS - 1:
                                nc.vector.tensor_scalar_add(
                                    dst[:, sq * QW:(sq + 1) * QW], st8["ps"][:],
                                    b_sb[:, hp:hp + 1],
                                )

                        chain = [lambda c=c, u=unit: u(c)
                                 for c in range(HCHUNKS)]
                        chains[(kind, sq)] = chain
                        units.extend(chain)
                return qt_t, kt_t, units, chains

            def emit_outproj_unit(st, half, hpc, st8):
                """One of three K=128 matmuls of the out-proj psum chain
                (st, half); hpc = head-pair chunk. Last chunk drains to
                SBUF and streams the result out."""
                if hpc == 0:
                    st8["ps"] = ps_pj.tile([128, 512], fp32, tag="pj", name="pso")
                nc.tensor.matmul(
                    st8["ps"][:, 0:384],
                    lhsT=combt_s[hpc][:, st * 128:(st + 1) * 128],
                    rhs=wo_sb[hpc][:, half * 384:(half + 1) * 384],
                    start=(hpc == 0), stop=(hpc == NHP - 1),
                )
                if hpc == NHP - 1:
                    ob = outsb.tile([128, 384], fp32, tag="ob")
                    nc.vector.tensor_copy(ob[:], st8["ps"][:, 0:384])
                    nc.sync.dma_start(
                        out_d[st * 128:(st + 1) * 128,
                              half * 384:(half + 1) * 384], ob[:])

            def outproj_units(sqs, min_sq):
                """(min_sq, deadline, unit) out-proj chains for the s-tiles
                inside query tiles `sqs` (needs all 3 head pairs' combT)."""
                units = []
                for sq in sqs:
                    for st in range(4 * sq, 4 * (sq + 1)):
                        for half in range(2):
                            st8 = {}
                            for hpc in range(NHP):
                                units.append((min_sq, None,
                                              lambda st=st, half=half,
                                              hpc=hpc, st8=st8:
                                              emit_outproj_unit(st, half, hpc, st8)))
                return units

            # ---- minimal preamble: only what the first two scores need ----
            qkts = [emit_qkt(0)]
            ch0 = qkts[0][3]
            for u in ch0[("kt", 0)] + ch0[("qt", 0)] + ch0[("kt", 1)]:
                u()

            # per-hp injection queues: (min_sq, deadline, emit_fn).
            # deadline = global slot index by whose lookahead-emission the
            # unit must have been EMITTED (its output is read by that
            # slot's scores lookahead or attn@V); None = no deadline.
            inject_q = {0: [], 1: [], 2: []}
            qkts.append(emit_qkt(1))
            q1ch = qkts[1][3]
            q1_main = [u for key in (("kt", 0), ("kt", 1), ("kt", 2), ("kt", 3),
                                     ("qt", 0), ("qt", 1))
                       for u in q1ch[key]]
            inject_q[0] = (
                [(0, kt, u) for kt in range(0, 2) for u in v_units(kt)]
                + [(0, 8, u) for u in ch0[("kt", 2)]]
                + [(0, kt, u) for kt in range(2, 4) for u in v_units(kt)]
                + [(0, 12, u) for u in ch0[("kt", 3)]]
                + [(0, kt, u) for kt in range(4, 6) for u in v_units(kt)]
                + [(0, 16, u) for u in ch0[("qt", 1)]]
                + [(0, kt, u) for kt in range(6, SKT) for u in v_units(kt)]
                + [(1, 32, u) for u in ch0[("qt", 2)]]
                + [(2, 48, u) for u in ch0[("qt", 3)]]
                + [(0, 64, u) for u in q1_main])

            slots = [(hp, sq, kt) for hp in range(NHP) for sq in range(SQT)
                     for kt in range(SKT)]

            def scores(hp, sq, kt):
                qt_t, kt_t = qkts[hp][0], qkts[hp][1]
                sc = ps_sc.tile([128, 1024], fp32, tag="sc", name="sc")
                nc.tensor.matmul(
                    sc[:, 0:512],
                    lhsT=kt_t[0:64, kt * 128:(kt + 1) * 128],
                    rhs=qt_t[0:64, sq * QW:(sq + 1) * QW],
                    start=True, stop=True,
                )
                nc.tensor.matmul(
                    sc[:, 512:1024],
                    lhsT=kt_t[64:128, kt * 128:(kt + 1) * 128],
                    rhs=qt_t[64:128, sq * QW:(sq + 1) * QW],
                    start=True, stop=True,
                )
                return sc

            cb_cur = None
            tail_tmpb = []
            sc_cur = scores(*slots[0])
            for i, (hp, sq, kt) in enumerate(slots):
                if kt == 0:
                    if sq == 0 and hp > 0:
                        # drain any leftover injected work of the previous hp
                        for _, _, u in inject_q[hp - 1]:
                            u()
                        inject_q[hp - 1] = []
                    # build hp-level injection queues lazily at hp start
                    if sq == 0 and hp == 1:
                        qkts.append(emit_qkt(2))
                        inject_q[1] = (
                            [(0, 96, u) for u in q1ch[("qt", 2)]]
                            + [(0, 112, u) for u in q1ch[("qt", 3)]]
                            + [(0, 128, u) for u in qkts[2][2]])
                    if sq == 0 and hp == 2:
                        inject_q[2] = (outproj_units([0], min_sq=1)
                                       + outproj_units([1], min_sq=2)
                                       + outproj_units([2], min_sq=3))
                    cb_a = ps_cb.tile([65, 512], fp32, tag="cb", name="cba")
                    cb_b = ps_cb.tile([65, 512], fp32, tag="cb", name="cbb")
                    cb_cur = (cb_a, cb_b)
                q = inject_q[hp]
                # forced pops: units whose output the upcoming lookahead
                # (slot i+1) or this slot's attn@V reads must be EMITTED
                # before those reads, or the tile deps are missed entirely.
                j = 0
                while j < len(q):
                    if q[j][1] is not None and q[j][1] <= i + 1:
                        q.pop(j)[2]()
                    else:
                        j += 1
                # lookahead scores for the next slot
                sc_nxt = scores(*slots[i + 1]) if i + 1 < len(slots) else None
                at = attnp.tile([128, 1024], bf16, tag="at")
                nc.scalar.activation(
                    at[:], sc_cur[:], AF.Exp,
                    bias=mask_sb[:, kt:kt + 1], scale=0.125,
                )
                # fill the PE exp-wait bubble with independent work
                # (scan past gated units so a blocked head doesn't starve
                # eligible work behind it)
                popped = 0
                if hp == 0:
                    max_pop = 7 if sq == 0 else 2
                elif hp == 1:
                    max_pop = 2
                else:
                    max_pop = 2
                j = 0
                while j < len(q) and popped < max_pop:
                    if q[j][0] <= sq:
                        q.pop(j)[2]()
                        popped += 1
                    else:
                        j += 1
                cb_a, cb_b = cb_cur
                nc.tensor.matmul(
                    cb_a[:],
                    lhsT=v_sb[kt][:, 2 * hp, :],
                    rhs=at[:, 0:512],
                    start=(kt == 0), stop=(kt == SKT - 1),
                )
                nc.tensor.matmul(
                    cb_b[:],
                    lhsT=v_sb[kt][:, 2 * hp + 1, :],
                    rhs=at[:, 512:1024],
                    start=(kt == 0), stop=(kt == SKT - 1),
                )
                sc_cur = sc_nxt
                if i == len(slots) - 1:
                    # keep the PE active through the serial tail normalize
                    # so the HAM clock gate stays at full rate for the
                    # tail out-proj matmuls
                    for _ in range(8):
                        dmy = ps_sc.tile([128, 1024], fp32, tag="sc",
                                         name="dmy")
                        nc.tensor.matmul(
                            dmy[:, 0:512],
                            lhsT=qkts[2][1][0:64, 0:128],
                            rhs=qkts[2][0][0:64, 0:512],
                            start=True, stop=True,
                        )
                if kt == SKT - 1:
                    # normalize: comb rows 0..63 / denom(row 64).
                    # Two quick copies free both PSUM banks before the
                    # slow recip/broadcast chains run.
                    last = (hp == NHP - 1 and sq == SQT - 1)
                    cbs_list = []
                    for cb in (cb_a, cb_b):
                        cbs = smallp.tile([65, 512], fp32, tag="cbs", name="cbs")
                        nc.vector.tensor_copy(cbs[:], cb[:])
                        cbs_list.append(cbs)
                    for half_b, cbs in enumerate(cbs_list):
                        rc0 = smallp.tile([1, 512], fp32, tag="rc0")
                        nc.sync.dma_start(rc0[:], cbs[64:65, :])
                        rc1 = smallp.tile([1, 512], fp32, tag="rc1")
                        # approx recip is partition-0 only on HW
                        nc.vector.reciprocal_approx_fast(rc1[:], rc0[:])
                        if last:
                            # tail: broadcast via a PE ones-matmul (PE is
                            # idle here; skips the slow gpsimd broadcast)
                            bcp = ps_pj.tile([128, 512], fp32, tag="pj",
                                             name="bcp")
                            nc.tensor.matmul(bcp[0:64, :], lhsT=ones_f[:],
                                             rhs=rc1[:], start=True, stop=True)
                            bc_ap = bcp[0:64, :]
                        else:
                            bc = smallp.tile([64, 512], fp32, tag="bc")
                            nc.gpsimd.partition_broadcast(bc[:], rc1[:])
                            bc_ap = bc[:]
                        if half_b == 0:
                            nc.vector.tensor_mul(
                                combt_s[hp][0:64, sq * QW:(sq + 1) * QW],
                                cbs[0:64, :], bc_ap,
                            )
                        else:
                            tmpb = smallp.tile([64, 512], bf16, tag="tmpb")
                            nc.vector.tensor_mul(tmpb[:], cbs[0:64, :], bc_ap)
                            if last:
                                tail_tmpb.append(tmpb)
                            else:
                                nc.gpsimd.dma_start(
                                    combt_s[hp][64:128, sq * QW:(sq + 1) * QW],
                                    tmpb[:])

            # ---- tail: leftovers + out-proj of the last query tile.
            # The hp2 slice of the last tile skips the stacking hop: head A
            # reads combt_s[2] rows 0-63, head B reads the normalize temp
            # directly against a base-0 copy of Wo's head-5 rows. ----
            for hp in range(NHP):
                for _, _, u in inject_q[hp]:
                    u()
                inject_q[hp] = []
            tmpb3 = tail_tmpb[0]
            chain_i = 0
            for st in range(4 * (SQT - 1), 4 * SQT):
                for half in range(2):
                    # alternate PSUM pools for a 4-deep chain rotation
                    # (ps_sc is idle in the tail)
                    if chain_i % 2 == 0:
                        ps = ps_pj.tile([128, 512], fp32, tag="pj", name="pso")
                    else:
                        ps = ps_sc.tile([128, 1024], fp32, tag="sc", name="pso")
                    chain_i += 1
                    for hpc in range(2):
                        nc.tensor.matmul(
                            ps[:, 0:384],
                            lhsT=combt_s[hpc][:, st * 128:(st + 1) * 128],
                            rhs=wo_sb[hpc][:, half * 384:(half + 1) * 384],
                            start=(hpc == 0), stop=False,
                        )
                    nc.tensor.matmul(
                        ps[:, 0:384],
                        lhsT=combt_s[2][0:64, st * 128:(st + 1) * 128],
                        rhs=wo_sb[2][0:64, half * 384:(half + 1) * 384],
                        start=False, stop=False,
                    )
                    nc.tensor.matmul(
                        ps[:, 0:384],
                        lhsT=tmpb3[:, (st - 12) * 128:(st - 11) * 128],
                        rhs=wo_hp2b[:, half * 384:(half + 1) * 384],
                        start=False, stop=True,
                    )
                    ob = outsb.tile([128, 384], fp32, tag="ob")
                    nc.vector.tensor_copy(ob[:], ps[:, 0:384])
                    nc.sync.dma_start(
                        out_d[st * 128:(st + 1) * 128,
                              half * 384:(half + 1) * 384], ob[:])

    nc.compile()
    return nc


def _get_compiled():
    global _COMPILED
    if _COMPILED is None:
        _COMPILED = _build()
    return _COMPILED


def _prep_core_inputs(x, mask, Wq, bq, Wk, bk, Wv, bv, Wo, core):
    b, hg = core // 2, core % 2
    lo, hi = hg * NHL * HS, (hg + 1) * NHL * HS
    bf = ml_dtypes.bfloat16
    return {
        "xt": np.ascontiguousarray(x[b].T).astype(bf),
        "wq": np.ascontiguousarray(Wq[:, lo:hi]).astype(bf),
        "wk": np.ascontiguousarray(Wk[:, lo:hi]).astype(bf),
        "wv": np.ascontiguousarray(Wv[:, lo:hi]).astype(bf),
        "wo": np.ascontiguousarray(Wo[lo:hi, :]).astype(bf),
        "bq": np.ascontiguousarray(bq[lo:hi].reshape(NHP, 128).T).astype(np.float32),
        "bk": np.ascontiguousarray(bk[lo:hi].reshape(NHP, 128).T).astype(np.float32),
        "bv": np.tile(bv[lo:hi][None, :], (128, 1)).astype(np.float32),
        "mask": np.ascontiguousarray(
            mask[b, 0, 0].reshape(SKT, 128).T).astype(np.float32),
    }


def kernel(x, additive_attention_mask, Wq, bq, Wk, bk, Wv, bv, Wo, bo):
    from concourse import bass2jax

    x = np.asarray(x, dtype=np.float32)
    mask = np.asarray(additive_attention_mask, dtype=np.float32)
    args = [np.asarray(a, dtype=np.float32) for a in (Wq, bq, Wk, bk, Wv, bv, Wo)]
    Wq, bq, Wk, bk, Wv, bv, Wo = args
    bo = np.asarray(bo, dtype=np.float32)

    nc = _get_compiled()
    in_maps = [
        _prep_core_inputs(x, mask, Wq, bq, Wk, bk, Wv, bv, Wo, c)
        for c in range(N_CORES)
    ]
    results = bass2jax.run_bass_via_pjrt(nc, in_maps, n_cores=N_CORES)

    out = np.empty((B, S, H), dtype=np.float32)
    for b in range(B):
        out[b] = results[2 * b]["out"] + results[2 * b + 1]["out"] + bo
    return out
